# revision 43
# baseline (speedup 1.0000x reference)
"""Trainium2 Bass kernel for BatchPrototypeHead (segment_reduce).

Math (reference):
    q = relu(query @ W1.T + b1) @ W2.T + b2          (B, d)
    s = relu(support @ W1.T + b1) @ W2.T + b2        (S, d)
    protos[c] = mean of s rows with label c (0 if empty)
    out[b, c] = -||q_b - protos_c||^2

Kernel strategy (8 NeuronCores, SPMD):
  - Shard query (B) and support (S) over the 8 cores.
  - Support side per core: h = relu(x @ W1.T) with rows on partitions
    (x and W1 streamed in fp8e4: the class-mean averages out the
    quantization), then ONE fused segment-sum matmul per 128-row chunk:
        hsumT[c, 0:256] += onehot.T @ h,   hsumT[c, 256] += counts
    via lhsT = onehot[row, class] fp16, rhs = [h | ones] fp16 (128x257),
    PSUM-accumulated over all 64 chunks into a class-major [100, 257].
  - Cross-core reduction of the [100, 257] fp16 partial via an ncfw
    AllGather (a dummy AllGather is fired first, unconsumed, so the
    one-time ncfw wake/entry-barrier cost overlaps the support phase).
    The alternative exchange=rdma path does direct peer SBUF writes
    (remote_dma_broadcast, XOR slotting + monotonic-semaphore gates);
    it works but its SWDGE descriptor flood (16 lanes x 64 line-descs
    per call, idle lanes emit dummies, ~100ns/desc) costs ~44us fully
    exposed after the support phase, so ncfw wins end-to-end.
  - b2 cancels exactly in ||q - p||^2; b1 is applied on the query side
    via the free ACT bias, and on the support side via an extra
    accumulating matmul only when b1 != 0 (trace-time branch).
  - Prototypes (class-major): 8-way DVE tree-sum of the gathered
    partials, scale rows by 1/max(cnt,1) (per-partition scalar),
    transpose via 2 identity matmuls, apply W2, keep 2*p.T fp16 for the
    distance matmul plus the [-1; -||p||^2] row.
  - Query projector in fp16 (full-rate PE, f32 PSUM) runs while the
    gather is in flight; distances as ONE PSUM accumulation per
    128-query chunk:
        G = qt.T @ (2 pT) + [qn; 1].T @ [-1; -pn] = 2 q.p - qn - pn
  - Expected rel err ~1.4e-3 (fp8 support stream dominates), well
    within the 2e-2 gate.
"""

import os
import numpy as np
import ml_dtypes
from contextlib import ExitStack

import concourse.bass as bass
import concourse.bacc as bacc
import concourse.mybir as mybir
import concourse.tile as tile
from concourse import bass_utils

F16 = mybir.dt.float16
F8 = mybir.dt.float8e4
F32 = mybir.dt.float32
I32 = mybir.dt.int32
AF = mybir.ActivationFunctionType
OP = mybir.AluOpType

B, S, D, C = 8192, 65536, 256, 100
NCORES = 8
BL, SL = B // NCORES, S // NCORES          # 1024 query rows, 8192 support rows
NCH = SL // 128                            # 64 support chunks / core
QCH = BL // 128                            # 8 query chunks / core
SLOT = 272                                 # padded 257 -> 272 (544B, 32B-aligned)



def build_program(b1_nonzero: bool, exchange: str):
    nc = bacc.Bacc("TRN2", target_bir_lowering=False, debug=False,
                   num_devices=NCORES, monotonic_sem_count=1)

    sync_wreg = None
    if exchange == "rdma":
        # arrival-wait target register on the SYNC engine (a gpsimd
        # event-semaphore wait on the remote-incremented sem resolved only
        # after ~10ms on HW; sync's hardware event semaphores do not).
        # Emitted OUTSIDE TileContext: Tile's lazy register handling drops
        # reg writes whose reader does not declare a register dependency.
        sync_wreg = nc.sync.alloc_register("rdma_wait_tgt")
        nc.sync.reg_mov(sync_wreg, (NCORES - 1) * 2)

    xt = nc.dram_tensor("xt_sup", [D, SL], F8, kind="ExternalInput").ap()
    lab = nc.dram_tensor("lab", [128, NCH], F32, kind="ExternalInput").ap()
    xqt = nc.dram_tensor("xqt", [D, BL], F16, kind="ExternalInput").ap()
    w1t = nc.dram_tensor("w1t", [D, D], F16, kind="ExternalInput").ap()
    w1t8 = nc.dram_tensor("w1t8", [D, D], F8, kind="ExternalInput").ap()
    w2t = nc.dram_tensor("w2t", [D, D], F16, kind="ExternalInput").ap()
    b1c = nc.dram_tensor("b1c", [D, 1], F32, kind="ExternalInput").ap()
    b1r = nc.dram_tensor("b1r", [1, D], F16, kind="ExternalInput").ap()
    out = nc.dram_tensor("out", [BL, C], F32, kind="ExternalOutput").ap()

    with tile.TileContext(nc) as tc, ExitStack() as ctx:
        sb = ctx.enter_context(tc.tile_pool(name="sb", bufs=1))
        dram = ctx.enter_context(tc.tile_pool(name="dram", bufs=1, space="DRAM"))
        hpool = ctx.enter_context(tc.tile_pool(name="hpool", bufs=3))
        ohpool = ctx.enter_context(tc.tile_pool(name="ohpool", bufs=3))
        opool = ctx.enter_context(tc.tile_pool(name="opool", bufs=3))

        # ---- receive slots for the cross-core exchange (written by PEERS
        # in rdma mode -- never memset slots 1..7, arrivals may be early)
        slots = sb.tile([128, NCORES, SLOT], F16, name="slots")
        partial = sb.tile([128, SLOT], F16, name="partial")
        nc.vector.memset(partial[:], 0.0)

        # dummy collective, fire-and-forget: wakes ncfw/TOPSP on every core
        # and absorbs the one-time wake/entry-barrier during the support
        # phase.  The tiny input DMA is load-bearing: it anchors the
        # trigger early in the schedule (with no dependency at all the
        # scheduler parks the trigger ~30us later, shifting the whole
        # ncfw chain).
        dmy_sb = sb.tile([8, 4], F32, name="dmy_sb")
        nc.vector.memset(dmy_sb[:], 0.0)
        dmy_in = dram.tile([8, 4], F32, name="dmy_in")
        dmy_out = dram.tile([NCORES * 8, 4], F32, name="dmy_out")
        nc.sync.dma_start(dmy_in[:], dmy_sb[:])
        nc.gpsimd.collective_compute(
            "AllGather", OP.bypass,
            replica_groups=[list(range(NCORES))],
            ins=[dmy_in.opt()], outs=[dmy_out.opt()])

        # ---- weights / constants for the support loop
        w1t_sb = [sb.tile([128, D], F16, name=f"w1t{k}") for k in range(2)]
        w1t8_sb = [sb.tile([128, D], F8, name=f"w1t8{k}") for k in range(2)]
        for k in range(2):
            nc.sync.dma_start(w1t_sb[k][:], w1t[k * 128:(k + 1) * 128, :])
            nc.sync.dma_start(w1t8_sb[k][:], w1t8[k * 128:(k + 1) * 128, :])
        lab_sb = sb.tile([128, NCH], F32, name="lab_sb")
        nc.sync.dma_start(lab_sb[:], lab[:])

        iota_i = sb.tile([128, C], I32, name="iota_i")
        nc.gpsimd.iota(iota_i[:], pattern=[[1, C]], base=0, channel_multiplier=0)
        iota_f = sb.tile([128, C], F32, name="iota_f")
        nc.vector.tensor_copy(iota_f[:], iota_i[:])
        iotac_i = sb.tile([128, 1], I32, name="iotac_i")
        nc.gpsimd.iota(iotac_i[:], pattern=[[1, 1]], base=0, channel_multiplier=1)
        iotac_f = sb.tile([128, 1], F32, name="iotac_f")
        nc.vector.tensor_copy(iotac_f[:], iotac_i[:])
        # identity[r, c] = (r == c), fp16, for the PE transpose
        ident = sb.tile([128, C], F16, name="ident")
        nc.vector.tensor_scalar(ident[:], iota_f[:], iotac_f[:], None,
                                OP.is_equal)
        if b1_nonzero:
            ones_row = sb.tile([1, 128], F16, name="ones_row")
            nc.vector.memset(ones_row[:], 1.0)
            b1r_sb = sb.tile([1, D], F16, name="b1r_sb")
            nc.sync.dma_start(b1r_sb[:], b1r[:])

        # ---- PE warm-up (HAM clock gate)
        scratch = sb.tile([128, 512], F16, name="scratch")
        nc.vector.memset(scratch[:], 0.5)
        with tc.tile_pool(name="pwarm", bufs=1, space="PSUM") as pwarm:
            warm_ps = pwarm.tile([128, 512], F32, name="warm_ps")
            for _ in range(4):
                nc.tensor.matmul(warm_ps[:], scratch[:, 0:128], scratch[:],
                                 start=True, stop=True, skip_group_check=True)

        # ---- support x.T: first group fine-grained so chunk 0 starts early
        xts = [[None] * 4 for _ in range(2)]
        xts0 = [[sb.tile([128, 512], F8, name=f"xts0_{k}_{s}")
                 for s in range(4)] for k in range(2)]
        for s in range(4):
            for k in range(2):
                nc.sync.dma_start(xts0[k][s][:],
                                  xt[k * 128:(k + 1) * 128,
                                     s * 512:(s + 1) * 512])
        for g in range(1, 4):
            for k in range(2):
                xts[k][g] = sb.tile([128, 2048], F8, name=f"xts{k}_{g}")
                nc.sync.dma_start(xts[k][g][:],
                                  xt[k * 128:(k + 1) * 128,
                                     g * 2048:(g + 1) * 2048])

        def x_slice(k, ci):
            g, sI = divmod(ci, 16)
            if g == 0:
                t, s2 = divmod(sI, 4)
                return xts0[k][t][:, s2 * 128:(s2 + 1) * 128]
            return xts[k][g][:, sI * 128:(sI + 1) * 128]

        # ---- query-side loads (consumed after the support phase)
        xqt_sb = [sb.tile([128, BL], F16, name=f"xqt{k}") for k in range(2)]
        w2t_sb = [sb.tile([128, D], F16, name=f"w2t{k}") for k in range(2)]
        b1c_sb = [sb.tile([128, 1], F32, name=f"b1c{k}") for k in range(2)]
        for k in range(2):
            nc.sync.dma_start(xqt_sb[k][:], xqt[k * 128:(k + 1) * 128, :])
            nc.sync.dma_start(w2t_sb[k][:], w2t[k * 128:(k + 1) * 128, :])
            nc.sync.dma_start(b1c_sb[k][:], b1c[k * 128:(k + 1) * 128, :])

        # constants for the augmented [2, *] matmul rows (partition-dim
        # writes must start at multiples of 32, so rows are built via
        # accumulating matmuls instead of direct row writes)
        oc2 = sb.tile([128, 2], F16, name="oc2")       # col0=1, col1=0
        nc.vector.memset(oc2[:, 0:1], 1.0)
        nc.vector.memset(oc2[:, 1:2], 0.0)
        zm1 = sb.tile([128, 2], F16, name="zm1")       # col0=0, col1=-1
        nc.vector.memset(zm1[:, 0:1], 0.0)
        nc.vector.memset(zm1[:, 1:2], -1.0)
        e01 = sb.tile([1, 2], F16, name="e01")         # (0, 1)
        nc.vector.memset(e01[:, 0:1], 0.0)
        nc.vector.memset(e01[:, 1:2], 1.0)
        em1 = sb.tile([1, 2], F16, name="em1")         # (-1, 0)
        nc.vector.memset(em1[:, 0:1], -1.0)
        nc.vector.memset(em1[:, 1:2], 0.0)
        ones_c = sb.tile([1, C], F16, name="ones_c")
        nc.vector.memset(ones_c[:], 1.0)
        ones_q = sb.tile([1, BL], F16, name="ones_q")
        nc.vector.memset(ones_q[:], 1.0)

        # ================= support phase =================
        if exchange == "rdma":
            rsem = nc.monotonic_semaphore(0).sem()
            lsem = nc.alloc_semaphore("rdma_local")
            tsem = nc.alloc_semaphore("rdma_trig")
        with tc.tile_pool(name="ph", bufs=3, space="PSUM") as ph, \
             tc.tile_pool(name="pacc", bufs=1, space="PSUM") as pacc:
            hsumT_ps = pacc.tile([C, 257], F32, name="hsumT_ps")
            for ci in range(NCH):
                h_ps = ph.tile([128, D], F32, name="h_ps")
                nc.tensor.matmul(h_ps[:], x_slice(0, ci), w1t8_sb[0][:],
                                 start=True, stop=False)
                nc.tensor.matmul(h_ps[:], x_slice(1, ci), w1t8_sb[1][:],
                                 start=False, stop=not b1_nonzero)
                if b1_nonzero:
                    nc.tensor.matmul(h_ps[:], ones_row[:], b1r_sb[:],
                                     start=False, stop=True)
                h_sb = hpool.tile([128, 257], F16, name="h_sb")
                nc.vector.memset(h_sb[:, 256:257], 1.0)
                nc.scalar.activation(h_sb[:, 0:256], h_ps[:], AF.Relu)
                oh = ohpool.tile([128, C], F16, name="oh")
                nc.vector.tensor_scalar(oh[:], iota_f[:], lab_sb[:, ci:ci + 1],
                                        None, OP.is_equal)
                nc.tensor.matmul(hsumT_ps[:], oh[:], h_sb[:],
                                 start=ci == 0, stop=ci == NCH - 1)
            # partial[0:100, 0:257] = [class sums | counts], fp16
            nc.scalar.copy(partial[0:100, 0:257], hsumT_ps[:])

        # ========== cross-core reduce: one-shot XOR-slotted exchange =====
        # call j sends the partial to tpb own^j, landing in the receiver's
        # slot j; the 7 calls are spread over the 4 SWDGE queues so the
        # descriptor generation (the cost driver: 16 lanes x 64 line-descs
        # per call, idle lanes emit dummies) runs in parallel.
        hsum_f = sb.tile([128, SLOT], F32, name="hsum_f")
        if exchange == "rdma":
            for j in range(1, NCORES):
                rd = [None] * NCORES
                rd[j] = (0, j)
                nc.gpsimd.remote_dma_broadcast(
                    slots[:, j:j + 1, :].squeeze(1), partial[:],
                    remote_sem=rsem, local_sem=lsem, rdests=rd)
            nc.gpsimd.trigger_dma(count=None).then_inc(tsem, 1)
            # arrival gate on SYNC (pinned after our own trigger via tsem
            # so the scheduler cannot hoist it ahead of the sends)
            stage_sem = nc.alloc_semaphore("rdma_st")
            ws = nc.sync.wait_ge(rsem, sync_wreg)
            ws.wait_op(tsem, 1, "sem-ge")
            ws.then_inc(stage_sem, 1)
            nc.vector.tensor_copy(slots[:, 0:1, :].squeeze(1), partial[:])
        else:
            GR = 104                       # gathered rows: 100 classes + pad
            arin = dram.tile([GR, SLOT], F16, name="arin")
            arout = dram.tile([NCORES * GR, SLOT], F16, name="arout")
            nc.sync.dma_start(arin[:], partial[0:GR, :])
            nc.gpsimd.collective_compute(
                "AllGather", OP.bypass,
                replica_groups=[list(range(NCORES))],
                ins=[arin.opt()], outs=[arout.opt()])
            ar_view = arout.rearrange("(r p) j -> p r j", p=GR)
            nc.sync.dma_start(slots[0:GR, :, :], ar_view)

        # 8-way tree sum on DVE
        a4 = sb.tile([128, 4, SLOT], F32, name="a4")
        add1 = nc.vector.tensor_tensor(a4[:], slots[:, 0:4, :],
                                       slots[:, 4:8, :], OP.add)
        if exchange == "rdma":
            add1.wait_op(stage_sem, 1, "sem-ge")
        a2v = sb.tile([128, 2, SLOT], F32, name="a2v")
        nc.vector.tensor_tensor(a2v[:], a4[:, 0:2, :], a4[:, 2:4, :], OP.add)
        nc.vector.tensor_tensor(hsum_f[:], a2v[:, 0:1, :].squeeze(1),
                                a2v[:, 1:2, :].squeeze(1), OP.add)

        # ================= query projector (runs while exchange is in
        # flight; also keeps the PE clock warm) =================
        hq_sb = [sb.tile([128, BL], F16, name=f"hq{k}") for k in range(2)]
        qt_sb = [sb.tile([128, BL], F16, name=f"qt{k}") for k in range(2)]
        qsq_sb = [sb.tile([128, BL], F16, name=f"qsq{k}") for k in range(2)]
        qaug = sb.tile([2, BL], F16, name="qaug")
        with tc.tile_pool(name="pq", bufs=2, space="PSUM") as pq:
            for qb in range(2):
                qsl = slice(qb * 512, (qb + 1) * 512)
                for mc in range(2):
                    msl = slice(mc * 128, (mc + 1) * 128)
                    hq_ps = pq.tile([128, 512], F32, name="hq_ps")
                    for kc in range(2):
                        nc.tensor.matmul(hq_ps[:], w1t_sb[kc][:, msl],
                                         xqt_sb[kc][:, qsl],
                                         start=kc == 0, stop=kc == 1)
                    nc.scalar.activation(hq_sb[mc][:, qsl], hq_ps[:], AF.Relu,
                                         bias=b1c_sb[mc][:])
                for mc in range(2):
                    msl = slice(mc * 128, (mc + 1) * 128)
                    qt_ps = pq.tile([128, 512], F32, name="qt_ps")
                    for kc in range(2):
                        nc.tensor.matmul(qt_ps[:], w2t_sb[kc][:, msl],
                                         hq_sb[kc][:, qsl],
                                         start=kc == 0, stop=kc == 1)
                    nc.scalar.copy(qt_sb[mc][:, qsl], qt_ps[:])
                    nc.scalar.square(qsq_sb[mc][:, qsl], qt_ps[:])
                # qaug rows: 0 = ||q||^2, 1 = ones
                qn_ps = pq.tile([2, 512], F32, name="qn_ps")
                for kc in range(2):
                    nc.tensor.matmul(qn_ps[:], oc2[:], qsq_sb[kc][:, qsl],
                                     start=kc == 0, stop=False)
                nc.tensor.matmul(qn_ps[:], e01[:], ones_q[:, qsl],
                                 start=False, stop=True)
                nc.scalar.copy(qaug[:, qsl], qn_ps[:])

        # ---- modest PE keep-warm filler during the exchange wait
        with tc.tile_pool(name="pfill", bufs=1, space="PSUM") as pfill:
            fill_ps = pfill.tile([128, 512], F32, name="fill_ps")
            for _ in range(16):
                nc.tensor.matmul(fill_ps[:], scratch[:, 0:128], scratch[:],
                                 start=True, stop=True, skip_group_check=True)

        # ================= prototypes =================
        cmax = sb.tile([128, 1], F32, name="cmax")
        inv = sb.tile([128, 1], F32, name="inv")
        hs_sc = sb.tile([128, D], F16, name="hs_sc")
        hsF_sb = [sb.tile([128, C], F16, name=f"hsF{k}") for k in range(2)]
        p2t_sb = [sb.tile([128, C], F16, name=f"p2t{k}") for k in range(2)]
        p2sq_sb = [sb.tile([128, C], F16, name=f"p2sq{k}") for k in range(2)]
        paug = sb.tile([2, C], F16, name="paug")
        with tc.tile_pool(name="pp", bufs=2, space="PSUM") as pp:
            nc.vector.tensor_scalar_max(cmax[0:100, :],
                                        hsum_f[0:100, 256:257], 1.0)
            nc.vector.reciprocal(inv[0:100, :], cmax[0:100, :])
            nc.vector.tensor_scalar(hs_sc[0:100, :], hsum_f[0:100, 0:256],
                                    inv[0:100, :], None, OP.mult)
            for k in range(2):
                tp_ps = pp.tile([128, C], F32, name="tp_ps")
                nc.tensor.matmul(tp_ps[:],
                                 hs_sc[0:100, k * 128:(k + 1) * 128],
                                 ident[0:100, :], start=True, stop=True)
                nc.scalar.copy(hsF_sb[k][:], tp_ps[:])
            for mc in range(2):
                msl = slice(mc * 128, (mc + 1) * 128)
                pt_ps = pp.tile([128, C], F32, name="pt_ps")
                for kc in range(2):
                    nc.tensor.matmul(pt_ps[:], w2t_sb[kc][:, msl],
                                     hsF_sb[kc][:],
                                     start=kc == 0, stop=kc == 1)
                nc.scalar.activation(p2t_sb[mc][:], pt_ps[:], AF.Copy,
                                     scale=2.0)
                nc.scalar.square(p2sq_sb[mc][:], pt_ps[:])
            # paug rows: 0 = -ones, 1 = -||p||^2
            pn_ps = pp.tile([2, C], F32, name="pn_ps")
            for mc in range(2):
                nc.tensor.matmul(pn_ps[:], zm1[:], p2sq_sb[mc][:],
                                 start=mc == 0, stop=False)
            nc.tensor.matmul(pn_ps[:], em1[:], ones_c[:],
                             start=False, stop=True)
            nc.scalar.copy(paug[:], pn_ps[:])

        # ================= distances =================
        with tc.tile_pool(name="pg", bufs=3, space="PSUM") as pg:
            for ci in range(QCH):
                csl = slice(ci * 128, (ci + 1) * 128)
                g_ps = pg.tile([128, C], F32, name="g_ps")
                nc.tensor.matmul(g_ps[:], qt_sb[0][:, csl], p2t_sb[0][:],
                                 start=True, stop=False)
                nc.tensor.matmul(g_ps[:], qt_sb[1][:, csl], p2t_sb[1][:],
                                 start=False, stop=False)
                nc.tensor.matmul(g_ps[:], qaug[:, csl], paug[:],
                                 start=False, stop=True)
                o_sb = opool.tile([128, C], F32, name="o_sb")
                nc.scalar.copy(o_sb[:], g_ps[:])
                nc.sync.dma_start(out[csl, :], o_sb[:])

    nc.compile()
    return nc


def make_in_maps(query, support_feats, support_labels, W1, b1, W2, b2):
    q = np.ascontiguousarray(np.asarray(query, dtype=np.float32))
    x = np.asarray(support_feats, dtype=np.float32)
    labels = np.asarray(support_labels).astype(np.int64)
    W1 = np.asarray(W1, dtype=np.float32)
    b1 = np.asarray(b1, dtype=np.float32)
    W2 = np.asarray(W2, dtype=np.float32)

    w1t_h = np.ascontiguousarray(W1.T).astype(np.float16)
    w1t8_h = np.ascontiguousarray(W1.T).astype(ml_dtypes.float8_e4m3fn)
    w2t_h = np.ascontiguousarray(W2.T).astype(np.float16)
    b1col = np.ascontiguousarray(b1.reshape(D, 1))
    b1row = np.ascontiguousarray(b1.reshape(1, D)).astype(np.float16)

    in_maps = []
    for c in range(NCORES):
        xs = x[c * SL:(c + 1) * SL]
        ls = labels[c * SL:(c + 1) * SL]
        xqs = q[c * BL:(c + 1) * BL]
        in_maps.append({
            "xt_sup": np.ascontiguousarray(xs.T).astype(
                ml_dtypes.float8_e4m3fn),
            "lab": np.ascontiguousarray(
                ls.reshape(NCH, 128).T.astype(np.float32)),
            "xqt": np.ascontiguousarray(xqs.T).astype(np.float16),
            "w1t": w1t_h,
            "w1t8": w1t8_h,
            "w2t": w2t_h,
            "b1c": b1col,
            "b1r": b1row,
        })
    return in_maps


EXCHANGE = os.environ.get("PROTO_EXCHANGE", "ccag")

_cached = {}


def _get_program(b1_nonzero: bool, exchange: str = None):
    key = (bool(b1_nonzero), exchange or EXCHANGE)
    if key not in _cached:
        _cached[key] = build_program(*key)
    return _cached[key]


def kernel(query, support_feats, support_labels, W1, b1, W2, b2,
           **run_kwargs):
    b1_nonzero = bool(np.any(np.asarray(b1)))
    nc = _get_program(b1_nonzero)
    in_maps = make_in_maps(query, support_feats, support_labels,
                           W1, b1, W2, b2)
    res = bass_utils.run_bass_kernel_spmd(
        nc, in_maps, core_ids=list(range(NCORES)), **run_kwargs)
    out = np.concatenate([res.results[c]["out"] for c in range(NCORES)],
                         axis=0)
    return out.astype(np.float32, copy=False)


if __name__ == "__main__":
    import sys
    sys.path.insert(0, "/root/problem")
    from reference import setup_inputs
    inputs = {k: np.asarray(v) for k, v in setup_inputs().items()}
    o = kernel(**inputs)
    print("out", o.shape, o.dtype, o[:2, :4])


# revision 44
# speedup vs baseline: 1.6233x; 1.6233x over previous
"""Trainium2 Bass kernel for BatchPrototypeHead (segment_reduce).

Math (reference):
    q = relu(query @ W1.T + b1) @ W2.T + b2          (B, d)
    s = relu(support @ W1.T + b1) @ W2.T + b2        (S, d)
    protos[c] = mean of s rows with label c (0 if empty)
    out[b, c] = -||q_b - protos_c||^2

Kernel strategy (8 NeuronCores, SPMD):
  - Shard query (B) and support (S) over the 8 cores.
  - Support side per core: h = relu(x @ W1.T) with rows on partitions
    (x and W1 streamed in fp8e4: the class-mean averages out the
    quantization), then ONE fused segment-sum matmul per 128-row chunk:
        hsumT[c, 0:256] += onehot.T @ h,   hsumT[c, 256] += counts
    via lhsT = onehot[row, class] fp16, rhs = [h | ones] fp16 (128x257),
    PSUM-accumulated over all 64 chunks into a class-major [100, 257].
  - Cross-core reduction of the [100, 257] fp16 partial via an ncfw
    AllGather (a dummy AllGather is fired first, unconsumed, so the
    one-time ncfw wake/entry-barrier cost overlaps the support phase).
    The alternative exchange=rdma path does direct peer SBUF writes
    (remote_dma_broadcast, XOR slotting + monotonic-semaphore gates);
    it works but its SWDGE descriptor flood (16 lanes x 64 line-descs
    per call, idle lanes emit dummies, ~100ns/desc) costs ~44us fully
    exposed after the support phase, so ncfw wins end-to-end.
  - b2 cancels exactly in ||q - p||^2; b1 is applied on the query side
    via the free ACT bias, and on the support side via an extra
    accumulating matmul only when b1 != 0 (trace-time branch).
  - Prototypes (class-major): 8-way DVE tree-sum of the gathered
    partials, scale rows by 1/max(cnt,1) (per-partition scalar),
    transpose via 2 identity matmuls, apply W2, keep 2*p.T fp16 for the
    distance matmul plus the [-1; -||p||^2] row.
  - Query projector in fp16 (full-rate PE, f32 PSUM) runs while the
    gather is in flight; distances as ONE PSUM accumulation per
    128-query chunk:
        G = qt.T @ (2 pT) + [qn; 1].T @ [-1; -pn] = 2 q.p - qn - pn
  - Expected rel err ~1.4e-3 (fp8 support stream dominates), well
    within the 2e-2 gate.
"""

import os
import numpy as np
import ml_dtypes
from contextlib import ExitStack

import concourse.bass as bass
import concourse.bacc as bacc
import concourse.mybir as mybir
import concourse.tile as tile
from concourse import bass_utils

F16 = mybir.dt.float16
F8 = mybir.dt.float8e4
F32 = mybir.dt.float32
I32 = mybir.dt.int32
AF = mybir.ActivationFunctionType
OP = mybir.AluOpType

B, S, D, C = 8192, 65536, 256, 100
NCORES = 8
BL, SL = B // NCORES, S // NCORES          # 1024 query rows, 8192 support rows
NCH = SL // 128                            # 64 support chunks / core
QCH = BL // 128                            # 8 query chunks / core
SLOT = 272                                 # padded 257 -> 272 (544B, 32B-aligned)



def build_program(b1_nonzero: bool, exchange: str):
    nc = bacc.Bacc("TRN2", target_bir_lowering=False, debug=False,
                   num_devices=NCORES, monotonic_sem_count=1)

    sync_wreg = None
    if exchange == "rdma":
        # arrival-wait target register on the SYNC engine (a gpsimd
        # event-semaphore wait on the remote-incremented sem resolved only
        # after ~10ms on HW; sync's hardware event semaphores do not).
        # Emitted OUTSIDE TileContext: Tile's lazy register handling drops
        # reg writes whose reader does not declare a register dependency.
        sync_wreg = nc.sync.alloc_register("rdma_wait_tgt")
        nc.sync.reg_mov(sync_wreg, (NCORES - 1) * 2)

    xt = nc.dram_tensor("xt_sup", [D, SL], F8, kind="ExternalInput").ap()
    lab = nc.dram_tensor("lab", [128, NCH], F32, kind="ExternalInput").ap()
    xqt = nc.dram_tensor("xqt", [D, BL], F16, kind="ExternalInput").ap()
    w1t = nc.dram_tensor("w1t", [D, D], F16, kind="ExternalInput").ap()
    w1t8 = nc.dram_tensor("w1t8", [D, D], F8, kind="ExternalInput").ap()
    w2t = nc.dram_tensor("w2t", [D, D], F16, kind="ExternalInput").ap()
    b1c = nc.dram_tensor("b1c", [D, 1], F32, kind="ExternalInput").ap()
    b1r = nc.dram_tensor("b1r", [1, D], F16, kind="ExternalInput").ap()
    out = nc.dram_tensor("out", [BL, C], F32, kind="ExternalOutput").ap()

    with tile.TileContext(nc) as tc, ExitStack() as ctx:
        sb = ctx.enter_context(tc.tile_pool(name="sb", bufs=1))
        dram = ctx.enter_context(tc.tile_pool(name="dram", bufs=1, space="DRAM"))
        hpool = ctx.enter_context(tc.tile_pool(name="hpool", bufs=3))
        ohpool = ctx.enter_context(tc.tile_pool(name="ohpool", bufs=3))
        opool = ctx.enter_context(tc.tile_pool(name="opool", bufs=3))

        # ---- receive slots for the cross-core exchange (written by PEERS
        # in rdma mode -- never memset slots 1..7, arrivals may be early)
        slots = sb.tile([128, NCORES, SLOT], F16, name="slots")
        partial = sb.tile([128, SLOT], F16, name="partial")
        nc.vector.memset(partial[:], 0.0)

        # dummy collective, fire-and-forget: wakes ncfw/TOPSP on every core
        # and absorbs the one-time wake/entry-barrier during the support
        # phase.  The tiny input DMA is load-bearing: it anchors the
        # trigger early in the schedule (with no dependency at all the
        # scheduler parks the trigger ~30us later, shifting the whole
        # ncfw chain).
        dmy_sb = sb.tile([8, 4], F32, name="dmy_sb")
        nc.vector.memset(dmy_sb[:], 0.0)
        dmy_in = dram.tile([8, 4], F32, name="dmy_in")
        dmy_out = dram.tile([NCORES * 8, 4], F32, name="dmy_out")
        nc.sync.dma_start(dmy_in[:], dmy_sb[:])
        nc.gpsimd.collective_compute(
            "AllGather", OP.bypass,
            replica_groups=[list(range(NCORES))],
            ins=[dmy_in.opt()], outs=[dmy_out.opt()])

        # ---- weights / constants for the support loop
        w1t_sb = [sb.tile([128, D], F16, name=f"w1t{k}") for k in range(2)]
        w1t8_sb = [sb.tile([128, D], F8, name=f"w1t8{k}") for k in range(2)]
        for k in range(2):
            nc.sync.dma_start(w1t_sb[k][:], w1t[k * 128:(k + 1) * 128, :])
            nc.sync.dma_start(w1t8_sb[k][:], w1t8[k * 128:(k + 1) * 128, :])
        lab_sb = sb.tile([128, NCH], F32, name="lab_sb")
        nc.sync.dma_start(lab_sb[:], lab[:])

        iota_i = sb.tile([128, C], I32, name="iota_i")
        nc.gpsimd.iota(iota_i[:], pattern=[[1, C]], base=0, channel_multiplier=0)
        iota_f = sb.tile([128, C], F32, name="iota_f")
        nc.vector.tensor_copy(iota_f[:], iota_i[:])
        iotac_i = sb.tile([128, 1], I32, name="iotac_i")
        nc.gpsimd.iota(iotac_i[:], pattern=[[1, 1]], base=0, channel_multiplier=1)
        iotac_f = sb.tile([128, 1], F32, name="iotac_f")
        nc.vector.tensor_copy(iotac_f[:], iotac_i[:])
        # identity[r, c] = (r == c), fp16, for the PE transpose
        ident = sb.tile([128, C], F16, name="ident")
        nc.vector.tensor_scalar(ident[:], iota_f[:], iotac_f[:], None,
                                OP.is_equal)
        if b1_nonzero:
            ones_row = sb.tile([1, 128], F16, name="ones_row")
            nc.vector.memset(ones_row[:], 1.0)
            b1r_sb = sb.tile([1, D], F16, name="b1r_sb")
            nc.sync.dma_start(b1r_sb[:], b1r[:])

        # ---- PE warm-up (HAM clock gate)
        scratch = sb.tile([128, 512], F16, name="scratch")
        nc.vector.memset(scratch[:], 0.5)
        with tc.tile_pool(name="pwarm", bufs=1, space="PSUM") as pwarm:
            warm_ps = pwarm.tile([128, 512], F32, name="warm_ps")
            for _ in range(4):
                nc.tensor.matmul(warm_ps[:], scratch[:, 0:128], scratch[:],
                                 start=True, stop=True, skip_group_check=True)

        # ---- support x.T: first group fine-grained so chunk 0 starts early
        xts = [[None] * 4 for _ in range(2)]
        xts0 = [[sb.tile([128, 512], F8, name=f"xts0_{k}_{s}")
                 for s in range(4)] for k in range(2)]
        for s in range(4):
            for k in range(2):
                nc.sync.dma_start(xts0[k][s][:],
                                  xt[k * 128:(k + 1) * 128,
                                     s * 512:(s + 1) * 512])
        for g in range(1, 4):
            for k in range(2):
                xts[k][g] = sb.tile([128, 2048], F8, name=f"xts{k}_{g}")
                nc.sync.dma_start(xts[k][g][:],
                                  xt[k * 128:(k + 1) * 128,
                                     g * 2048:(g + 1) * 2048])

        def x_slice(k, ci):
            g, sI = divmod(ci, 16)
            if g == 0:
                t, s2 = divmod(sI, 4)
                return xts0[k][t][:, s2 * 128:(s2 + 1) * 128]
            return xts[k][g][:, sI * 128:(sI + 1) * 128]

        # ---- query-side loads (consumed after the support phase)
        xqt_sb = [sb.tile([128, BL], F16, name=f"xqt{k}") for k in range(2)]
        w2t_sb = [sb.tile([128, D], F16, name=f"w2t{k}") for k in range(2)]
        b1c_sb = [sb.tile([128, 1], F32, name=f"b1c{k}") for k in range(2)]
        for k in range(2):
            nc.sync.dma_start(xqt_sb[k][:], xqt[k * 128:(k + 1) * 128, :])
            nc.sync.dma_start(w2t_sb[k][:], w2t[k * 128:(k + 1) * 128, :])
            nc.sync.dma_start(b1c_sb[k][:], b1c[k * 128:(k + 1) * 128, :])

        # constants for the augmented [2, *] matmul rows (partition-dim
        # writes must start at multiples of 32, so rows are built via
        # accumulating matmuls instead of direct row writes)
        oc2 = sb.tile([128, 2], F16, name="oc2")       # col0=1, col1=0
        nc.vector.memset(oc2[:, 0:1], 1.0)
        nc.vector.memset(oc2[:, 1:2], 0.0)
        zm1 = sb.tile([128, 2], F16, name="zm1")       # col0=0, col1=-1
        nc.vector.memset(zm1[:, 0:1], 0.0)
        nc.vector.memset(zm1[:, 1:2], -1.0)
        e01 = sb.tile([1, 2], F16, name="e01")         # (0, 1)
        nc.vector.memset(e01[:, 0:1], 0.0)
        nc.vector.memset(e01[:, 1:2], 1.0)
        em1 = sb.tile([1, 2], F16, name="em1")         # (-1, 0)
        nc.vector.memset(em1[:, 0:1], -1.0)
        nc.vector.memset(em1[:, 1:2], 0.0)
        ones_c = sb.tile([1, C], F16, name="ones_c")
        nc.vector.memset(ones_c[:], 1.0)
        ones_q = sb.tile([1, BL], F16, name="ones_q")
        nc.vector.memset(ones_q[:], 1.0)

        # ================= support phase =================
        if exchange == "rdma":
            rsem = nc.monotonic_semaphore(0).sem()
            lsem = nc.alloc_semaphore("rdma_local")
            tsem = nc.alloc_semaphore("rdma_trig")
        with tc.tile_pool(name="ph", bufs=3, space="PSUM") as ph, \
             tc.tile_pool(name="pacc", bufs=1, space="PSUM") as pacc:
            hsumT_ps = pacc.tile([C, 257], F32, name="hsumT_ps")
            for ci in range(NCH):
                h_ps = ph.tile([128, D], F32, name="h_ps")
                nc.tensor.matmul(h_ps[:], x_slice(0, ci), w1t8_sb[0][:],
                                 start=True, stop=False)
                nc.tensor.matmul(h_ps[:], x_slice(1, ci), w1t8_sb[1][:],
                                 start=False, stop=not b1_nonzero)
                if b1_nonzero:
                    nc.tensor.matmul(h_ps[:], ones_row[:], b1r_sb[:],
                                     start=False, stop=True)
                h_sb = hpool.tile([128, 257], F16, name="h_sb")
                nc.vector.memset(h_sb[:, 256:257], 1.0)
                nc.scalar.activation(h_sb[:, 0:256], h_ps[:], AF.Relu)
                oh = ohpool.tile([128, C], F16, name="oh")
                nc.vector.tensor_scalar(oh[:], iota_f[:], lab_sb[:, ci:ci + 1],
                                        None, OP.is_equal)
                nc.tensor.matmul(hsumT_ps[:], oh[:], h_sb[:],
                                 start=ci == 0, stop=ci == NCH - 1)
            # partial[0:100, 0:257] = [class sums | counts], fp16
            nc.scalar.copy(partial[0:100, 0:257], hsumT_ps[:])

        # ========== cross-core reduce: one-shot XOR-slotted exchange =====
        # call j sends the partial to tpb own^j, landing in the receiver's
        # slot j; the 7 calls are spread over the 4 SWDGE queues so the
        # descriptor generation (the cost driver: 16 lanes x 64 line-descs
        # per call, idle lanes emit dummies) runs in parallel.
        hsum_f = sb.tile([128, SLOT], F32, name="hsum_f")
        if exchange == "rdma":
            for j in range(1, NCORES):
                rd = [None] * NCORES
                rd[j] = (0, j)
                nc.gpsimd.remote_dma_broadcast(
                    slots[:, j:j + 1, :].squeeze(1), partial[:],
                    remote_sem=rsem, local_sem=lsem, rdests=rd)
            nc.gpsimd.trigger_dma(count=None).then_inc(tsem, 1)
            # arrival gate on SYNC (pinned after our own trigger via tsem
            # so the scheduler cannot hoist it ahead of the sends)
            stage_sem = nc.alloc_semaphore("rdma_st")
            ws = nc.sync.wait_ge(rsem, sync_wreg)
            ws.wait_op(tsem, 1, "sem-ge")
            ws.then_inc(stage_sem, 1)
            nc.vector.tensor_copy(slots[:, 0:1, :].squeeze(1), partial[:])
        else:
            arin = dram.tile([128, SLOT], F16, name="arin")
            arout = dram.tile([NCORES * 128, SLOT], F16, name="arout")
            nc.sync.dma_start(arin[:], partial[:])
            nc.gpsimd.collective_compute(
                "AllGather", OP.bypass,
                replica_groups=[list(range(NCORES))],
                ins=[arin.opt()], outs=[arout.opt()])
            ar_view = arout.rearrange("(r p) j -> p r j", p=128)
            nc.sync.dma_start(slots[:], ar_view)

        # 8-way tree sum on DVE
        a4 = sb.tile([128, 4, SLOT], F32, name="a4")
        add1 = nc.vector.tensor_tensor(a4[:], slots[:, 0:4, :],
                                       slots[:, 4:8, :], OP.add)
        if exchange == "rdma":
            add1.wait_op(stage_sem, 1, "sem-ge")
        a2v = sb.tile([128, 2, SLOT], F32, name="a2v")
        nc.vector.tensor_tensor(a2v[:], a4[:, 0:2, :], a4[:, 2:4, :], OP.add)
        nc.vector.tensor_tensor(hsum_f[:], a2v[:, 0:1, :].squeeze(1),
                                a2v[:, 1:2, :].squeeze(1), OP.add)

        # ================= query projector (runs while exchange is in
        # flight; also keeps the PE clock warm) =================
        hq_sb = [sb.tile([128, BL], F16, name=f"hq{k}") for k in range(2)]
        qt_sb = [sb.tile([128, BL], F16, name=f"qt{k}") for k in range(2)]
        qsq_sb = [sb.tile([128, BL], F16, name=f"qsq{k}") for k in range(2)]
        qaug = sb.tile([2, BL], F16, name="qaug")
        with tc.tile_pool(name="pq", bufs=2, space="PSUM") as pq:
            for qb in range(2):
                qsl = slice(qb * 512, (qb + 1) * 512)
                for mc in range(2):
                    msl = slice(mc * 128, (mc + 1) * 128)
                    hq_ps = pq.tile([128, 512], F32, name="hq_ps")
                    for kc in range(2):
                        nc.tensor.matmul(hq_ps[:], w1t_sb[kc][:, msl],
                                         xqt_sb[kc][:, qsl],
                                         start=kc == 0, stop=kc == 1)
                    nc.scalar.activation(hq_sb[mc][:, qsl], hq_ps[:], AF.Relu,
                                         bias=b1c_sb[mc][:])
                for mc in range(2):
                    msl = slice(mc * 128, (mc + 1) * 128)
                    qt_ps = pq.tile([128, 512], F32, name="qt_ps")
                    for kc in range(2):
                        nc.tensor.matmul(qt_ps[:], w2t_sb[kc][:, msl],
                                         hq_sb[kc][:, qsl],
                                         start=kc == 0, stop=kc == 1)
                    nc.scalar.copy(qt_sb[mc][:, qsl], qt_ps[:])
                    nc.scalar.square(qsq_sb[mc][:, qsl], qt_ps[:])
                # qaug rows: 0 = ||q||^2, 1 = ones
                qn_ps = pq.tile([2, 512], F32, name="qn_ps")
                for kc in range(2):
                    nc.tensor.matmul(qn_ps[:], oc2[:], qsq_sb[kc][:, qsl],
                                     start=kc == 0, stop=False)
                nc.tensor.matmul(qn_ps[:], e01[:], ones_q[:, qsl],
                                 start=False, stop=True)
                nc.scalar.copy(qaug[:, qsl], qn_ps[:])

        # ---- modest PE keep-warm filler during the exchange wait
        with tc.tile_pool(name="pfill", bufs=1, space="PSUM") as pfill:
            fill_ps = pfill.tile([128, 512], F32, name="fill_ps")
            for _ in range(16):
                nc.tensor.matmul(fill_ps[:], scratch[:, 0:128], scratch[:],
                                 start=True, stop=True, skip_group_check=True)

        # ================= prototypes =================
        cmax = sb.tile([128, 1], F32, name="cmax")
        inv = sb.tile([128, 1], F32, name="inv")
        hs_sc = sb.tile([128, D], F16, name="hs_sc")
        hsF_sb = [sb.tile([128, C], F16, name=f"hsF{k}") for k in range(2)]
        p2t_sb = [sb.tile([128, C], F16, name=f"p2t{k}") for k in range(2)]
        p2sq_sb = [sb.tile([128, C], F16, name=f"p2sq{k}") for k in range(2)]
        paug = sb.tile([2, C], F16, name="paug")
        with tc.tile_pool(name="pp", bufs=2, space="PSUM") as pp:
            nc.vector.tensor_scalar_max(cmax[0:100, :],
                                        hsum_f[0:100, 256:257], 1.0)
            nc.vector.reciprocal(inv[0:100, :], cmax[0:100, :])
            nc.vector.tensor_scalar(hs_sc[0:100, :], hsum_f[0:100, 0:256],
                                    inv[0:100, :], None, OP.mult)
            for k in range(2):
                tp_ps = pp.tile([128, C], F32, name="tp_ps")
                nc.tensor.matmul(tp_ps[:],
                                 hs_sc[0:100, k * 128:(k + 1) * 128],
                                 ident[0:100, :], start=True, stop=True)
                nc.scalar.copy(hsF_sb[k][:], tp_ps[:])
            for mc in range(2):
                msl = slice(mc * 128, (mc + 1) * 128)
                pt_ps = pp.tile([128, C], F32, name="pt_ps")
                for kc in range(2):
                    nc.tensor.matmul(pt_ps[:], w2t_sb[kc][:, msl],
                                     hsF_sb[kc][:],
                                     start=kc == 0, stop=kc == 1)
                nc.scalar.activation(p2t_sb[mc][:], pt_ps[:], AF.Copy,
                                     scale=2.0)
                nc.scalar.square(p2sq_sb[mc][:], pt_ps[:])
            # paug rows: 0 = -ones, 1 = -||p||^2
            pn_ps = pp.tile([2, C], F32, name="pn_ps")
            for mc in range(2):
                nc.tensor.matmul(pn_ps[:], zm1[:], p2sq_sb[mc][:],
                                 start=mc == 0, stop=False)
            nc.tensor.matmul(pn_ps[:], em1[:], ones_c[:],
                             start=False, stop=True)
            nc.scalar.copy(paug[:], pn_ps[:])

        # ================= distances =================
        with tc.tile_pool(name="pg", bufs=3, space="PSUM") as pg:
            for ci in range(QCH):
                csl = slice(ci * 128, (ci + 1) * 128)
                g_ps = pg.tile([128, C], F32, name="g_ps")
                nc.tensor.matmul(g_ps[:], qt_sb[0][:, csl], p2t_sb[0][:],
                                 start=True, stop=False)
                nc.tensor.matmul(g_ps[:], qt_sb[1][:, csl], p2t_sb[1][:],
                                 start=False, stop=False)
                nc.tensor.matmul(g_ps[:], qaug[:, csl], paug[:],
                                 start=False, stop=True)
                o_sb = opool.tile([128, C], F32, name="o_sb")
                nc.scalar.copy(o_sb[:], g_ps[:])
                nc.sync.dma_start(out[csl, :], o_sb[:])

    nc.compile()
    return nc


def make_in_maps(query, support_feats, support_labels, W1, b1, W2, b2):
    q = np.ascontiguousarray(np.asarray(query, dtype=np.float32))
    x = np.asarray(support_feats, dtype=np.float32)
    labels = np.asarray(support_labels).astype(np.int64)
    W1 = np.asarray(W1, dtype=np.float32)
    b1 = np.asarray(b1, dtype=np.float32)
    W2 = np.asarray(W2, dtype=np.float32)

    w1t_h = np.ascontiguousarray(W1.T).astype(np.float16)
    w1t8_h = np.ascontiguousarray(W1.T).astype(ml_dtypes.float8_e4m3fn)
    w2t_h = np.ascontiguousarray(W2.T).astype(np.float16)
    b1col = np.ascontiguousarray(b1.reshape(D, 1))
    b1row = np.ascontiguousarray(b1.reshape(1, D)).astype(np.float16)

    in_maps = []
    for c in range(NCORES):
        xs = x[c * SL:(c + 1) * SL]
        ls = labels[c * SL:(c + 1) * SL]
        xqs = q[c * BL:(c + 1) * BL]
        in_maps.append({
            "xt_sup": np.ascontiguousarray(xs.T).astype(
                ml_dtypes.float8_e4m3fn),
            "lab": np.ascontiguousarray(
                ls.reshape(NCH, 128).T.astype(np.float32)),
            "xqt": np.ascontiguousarray(xqs.T).astype(np.float16),
            "w1t": w1t_h,
            "w1t8": w1t8_h,
            "w2t": w2t_h,
            "b1c": b1col,
            "b1r": b1row,
        })
    return in_maps


EXCHANGE = os.environ.get("PROTO_EXCHANGE", "ccag")

_cached = {}


def _get_program(b1_nonzero: bool, exchange: str = None):
    key = (bool(b1_nonzero), exchange or EXCHANGE)
    if key not in _cached:
        _cached[key] = build_program(*key)
    return _cached[key]


def kernel(query, support_feats, support_labels, W1, b1, W2, b2,
           **run_kwargs):
    b1_nonzero = bool(np.any(np.asarray(b1)))
    nc = _get_program(b1_nonzero)
    in_maps = make_in_maps(query, support_feats, support_labels,
                           W1, b1, W2, b2)
    res = bass_utils.run_bass_kernel_spmd(
        nc, in_maps, core_ids=list(range(NCORES)), **run_kwargs)
    out = np.concatenate([res.results[c]["out"] for c in range(NCORES)],
                         axis=0)
    return out.astype(np.float32, copy=False)


if __name__ == "__main__":
    import sys
    sys.path.insert(0, "/root/problem")
    from reference import setup_inputs
    inputs = {k: np.asarray(v) for k, v in setup_inputs().items()}
    o = kernel(**inputs)
    print("out", o.shape, o.dtype, o[:2, :4])


# revision 46
# speedup vs baseline: 1.7080x; 1.0522x over previous
"""Trainium2 Bass kernel for BatchPrototypeHead (segment_reduce).

Math (reference):
    q = relu(query @ W1.T + b1) @ W2.T + b2          (B, d)
    s = relu(support @ W1.T + b1) @ W2.T + b2        (S, d)
    protos[c] = mean of s rows with label c (0 if empty)
    out[b, c] = -||q_b - protos_c||^2

Kernel strategy (8 NeuronCores, SPMD):
  - Shard query (B) and support (S) over the 8 cores.
  - Support side per core: h = relu(x @ W1.T) with rows on partitions
    (x and W1 streamed in fp8e4: the class-mean averages out the
    quantization), then ONE fused segment-sum matmul per 128-row chunk:
        hsumT[c, 0:256] += onehot.T @ h,   hsumT[c, 256] += counts
    via lhsT = onehot[row, class] fp16, rhs = [h | ones] fp16 (128x257),
    PSUM-accumulated over all 64 chunks into a class-major [100, 257].
  - Cross-core reduction of the [100, 257] fp16 partial via an ncfw
    AllGather (a dummy AllGather is fired first, unconsumed, so the
    one-time ncfw wake/entry-barrier cost overlaps the support phase).
    The alternative exchange=rdma path does direct peer SBUF writes
    (remote_dma_broadcast, XOR slotting + monotonic-semaphore gates);
    it works but its SWDGE descriptor flood (16 lanes x 64 line-descs
    per call, idle lanes emit dummies, ~100ns/desc) costs ~44us fully
    exposed after the support phase, so ncfw wins end-to-end.
  - b2 cancels exactly in ||q - p||^2; b1 is applied on the query side
    via the free ACT bias, and on the support side via an extra
    accumulating matmul only when b1 != 0 (trace-time branch).
  - Prototypes (class-major): 8-way DVE tree-sum of the gathered
    partials, scale rows by 1/max(cnt,1) (per-partition scalar),
    transpose via 2 identity matmuls, apply W2, keep 2*p.T fp16 for the
    distance matmul plus the [-1; -||p||^2] row.
  - Query projector in fp16 (full-rate PE, f32 PSUM) runs while the
    gather is in flight; distances as ONE PSUM accumulation per
    128-query chunk:
        G = qt.T @ (2 pT) + [qn; 1].T @ [-1; -pn] = 2 q.p - qn - pn
  - Expected rel err ~1.4e-3 (fp8 support stream dominates), well
    within the 2e-2 gate.
"""

import os
import numpy as np
import ml_dtypes
from contextlib import ExitStack

import concourse.bass as bass
import concourse.bacc as bacc
import concourse.mybir as mybir
import concourse.tile as tile
from concourse import bass_utils

F16 = mybir.dt.float16
F8 = mybir.dt.float8e4
F32 = mybir.dt.float32
I32 = mybir.dt.int32
AF = mybir.ActivationFunctionType
OP = mybir.AluOpType

B, S, D, C = 8192, 65536, 256, 100
NCORES = 8
BL, SL = B // NCORES, S // NCORES          # 1024 query rows, 8192 support rows
NCH = SL // 128                            # 64 support chunks / core
QCH = BL // 128                            # 8 query chunks / core
SLOT = 272                                 # padded 257 -> 272 (544B, 32B-aligned)



def build_program(b1_nonzero: bool, exchange: str):
    nc = bacc.Bacc("TRN2", target_bir_lowering=False, debug=False,
                   num_devices=NCORES, monotonic_sem_count=1)

    sync_wreg = None
    if exchange == "rdma":
        # arrival-wait target register on the SYNC engine (a gpsimd
        # event-semaphore wait on the remote-incremented sem resolved only
        # after ~10ms on HW; sync's hardware event semaphores do not).
        # Emitted OUTSIDE TileContext: Tile's lazy register handling drops
        # reg writes whose reader does not declare a register dependency.
        sync_wreg = nc.sync.alloc_register("rdma_wait_tgt")
        nc.sync.reg_mov(sync_wreg, (NCORES - 1) * 2)

    xt = nc.dram_tensor("xt_sup", [D, SL], F8, kind="ExternalInput").ap()
    lab = nc.dram_tensor("lab", [128, NCH], F32, kind="ExternalInput").ap()
    xqt = nc.dram_tensor("xqt", [D, BL], F16, kind="ExternalInput").ap()
    w1t = nc.dram_tensor("w1t", [D, D], F16, kind="ExternalInput").ap()
    w1t8 = nc.dram_tensor("w1t8", [D, D], F8, kind="ExternalInput").ap()
    w2t = nc.dram_tensor("w2t", [D, D], F16, kind="ExternalInput").ap()
    b1c = nc.dram_tensor("b1c", [D, 1], F32, kind="ExternalInput").ap()
    b1r = nc.dram_tensor("b1r", [1, D], F16, kind="ExternalInput").ap()
    out = nc.dram_tensor("out", [BL, C], F32, kind="ExternalOutput").ap()

    with tile.TileContext(nc) as tc, ExitStack() as ctx:
        sb = ctx.enter_context(tc.tile_pool(name="sb", bufs=1))
        dram = ctx.enter_context(tc.tile_pool(name="dram", bufs=1, space="DRAM"))
        hpool = ctx.enter_context(tc.tile_pool(name="hpool", bufs=3))
        ohpool = ctx.enter_context(tc.tile_pool(name="ohpool", bufs=3))
        opool = ctx.enter_context(tc.tile_pool(name="opool", bufs=3))

        # ---- receive slots for the cross-core exchange (written by PEERS
        # in rdma mode -- never memset slots 1..7, arrivals may be early)
        slots = sb.tile([128, NCORES, SLOT], F16, name="slots")
        partial = sb.tile([128, SLOT], F16, name="partial")
        nc.vector.memset(partial[:], 0.0)

        # dummy collective, fire-and-forget: wakes ncfw/TOPSP on every core
        # and absorbs the one-time wake/entry-barrier during the support
        # phase.  The tiny input DMA is load-bearing: it anchors the
        # trigger early in the schedule (with no dependency at all the
        # scheduler parks the trigger ~30us later, shifting the whole
        # ncfw chain).
        dmy_sb = sb.tile([8, 4], F32, name="dmy_sb")
        nc.vector.memset(dmy_sb[:], 0.0)
        dmy_in = dram.tile([8, 4], F32, name="dmy_in")
        dmy_out = dram.tile([NCORES * 8, 4], F32, name="dmy_out")
        nc.sync.dma_start(dmy_in[:], dmy_sb[:])
        nc.gpsimd.collective_compute(
            "AllGather", OP.bypass,
            replica_groups=[list(range(NCORES))],
            ins=[dmy_in.opt()], outs=[dmy_out.opt()])

        # ---- weights / constants for the support loop
        w1t_sb = [sb.tile([128, D], F16, name=f"w1t{k}") for k in range(2)]
        w1t8_sb = [sb.tile([128, D], F8, name=f"w1t8{k}") for k in range(2)]
        for k in range(2):
            nc.sync.dma_start(w1t_sb[k][:], w1t[k * 128:(k + 1) * 128, :])
            nc.sync.dma_start(w1t8_sb[k][:], w1t8[k * 128:(k + 1) * 128, :])
        lab_sb = sb.tile([128, NCH], F32, name="lab_sb")
        nc.sync.dma_start(lab_sb[:], lab[:])

        iota_i = sb.tile([128, C], I32, name="iota_i")
        nc.gpsimd.iota(iota_i[:], pattern=[[1, C]], base=0, channel_multiplier=0)
        iota_f = sb.tile([128, C], F32, name="iota_f")
        nc.vector.tensor_copy(iota_f[:], iota_i[:])
        iotac_i = sb.tile([128, 1], I32, name="iotac_i")
        nc.gpsimd.iota(iotac_i[:], pattern=[[1, 1]], base=0, channel_multiplier=1)
        iotac_f = sb.tile([128, 1], F32, name="iotac_f")
        nc.vector.tensor_copy(iotac_f[:], iotac_i[:])
        # identity[r, c] = (r == c), fp16, for the PE transpose
        ident = sb.tile([128, C], F16, name="ident")
        nc.vector.tensor_scalar(ident[:], iota_f[:], iotac_f[:], None,
                                OP.is_equal)
        if b1_nonzero:
            ones_row = sb.tile([1, 128], F16, name="ones_row")
            nc.vector.memset(ones_row[:], 1.0)
            b1r_sb = sb.tile([1, D], F16, name="b1r_sb")
            nc.sync.dma_start(b1r_sb[:], b1r[:])

        # ---- PE warm-up (HAM clock gate)
        scratch = sb.tile([128, 512], F16, name="scratch")
        nc.vector.memset(scratch[:], 0.5)
        with tc.tile_pool(name="pwarm", bufs=1, space="PSUM") as pwarm:
            warm_ps = pwarm.tile([128, 512], F32, name="warm_ps")
            for _ in range(4):
                nc.tensor.matmul(warm_ps[:], scratch[:, 0:128], scratch[:],
                                 start=True, stop=True, skip_group_check=True)

        # ---- support x.T: first group fine-grained so chunk 0 starts early
        xts = [[None] * 4 for _ in range(2)]
        xts0 = [[sb.tile([128, 512], F8, name=f"xts0_{k}_{s}")
                 for s in range(4)] for k in range(2)]
        for s in range(4):
            for k in range(2):
                nc.sync.dma_start(xts0[k][s][:],
                                  xt[k * 128:(k + 1) * 128,
                                     s * 512:(s + 1) * 512])
        for g in range(1, 4):
            for k in range(2):
                xts[k][g] = sb.tile([128, 2048], F8, name=f"xts{k}_{g}")
                nc.sync.dma_start(xts[k][g][:],
                                  xt[k * 128:(k + 1) * 128,
                                     g * 2048:(g + 1) * 2048])

        def x_slice(k, ci):
            g, sI = divmod(ci, 16)
            if g == 0:
                t, s2 = divmod(sI, 4)
                return xts0[k][t][:, s2 * 128:(s2 + 1) * 128]
            return xts[k][g][:, sI * 128:(sI + 1) * 128]

        # ---- query-side loads (consumed after the support phase)
        xqt_sb = [sb.tile([128, BL], F16, name=f"xqt{k}") for k in range(2)]
        w2t_sb = [sb.tile([128, D], F16, name=f"w2t{k}") for k in range(2)]
        b1c_sb = [sb.tile([128, 1], F32, name=f"b1c{k}") for k in range(2)]
        for k in range(2):
            nc.sync.dma_start(xqt_sb[k][:], xqt[k * 128:(k + 1) * 128, :])
            nc.sync.dma_start(w2t_sb[k][:], w2t[k * 128:(k + 1) * 128, :])
            nc.sync.dma_start(b1c_sb[k][:], b1c[k * 128:(k + 1) * 128, :])

        # constants for the augmented [2, *] matmul rows (partition-dim
        # writes must start at multiples of 32, so rows are built via
        # accumulating matmuls instead of direct row writes)
        oc2 = sb.tile([128, 2], F16, name="oc2")       # col0=1, col1=0
        nc.vector.memset(oc2[:, 0:1], 1.0)
        nc.vector.memset(oc2[:, 1:2], 0.0)
        zm1 = sb.tile([128, 2], F16, name="zm1")       # col0=0, col1=-1
        nc.vector.memset(zm1[:, 0:1], 0.0)
        nc.vector.memset(zm1[:, 1:2], -1.0)
        e01 = sb.tile([1, 2], F16, name="e01")         # (0, 1)
        nc.vector.memset(e01[:, 0:1], 0.0)
        nc.vector.memset(e01[:, 1:2], 1.0)
        em1 = sb.tile([1, 2], F16, name="em1")         # (-1, 0)
        nc.vector.memset(em1[:, 0:1], -1.0)
        nc.vector.memset(em1[:, 1:2], 0.0)
        ones_c = sb.tile([1, C], F16, name="ones_c")
        nc.vector.memset(ones_c[:], 1.0)
        ones_q = sb.tile([1, BL], F16, name="ones_q")
        nc.vector.memset(ones_q[:], 1.0)

        # ================= support phase =================
        if exchange == "rdma":
            rsem = nc.monotonic_semaphore(0).sem()
            lsem = nc.alloc_semaphore("rdma_local")
            tsem = nc.alloc_semaphore("rdma_trig")
        with tc.tile_pool(name="ph", bufs=3, space="PSUM") as ph, \
             tc.tile_pool(name="pacc", bufs=1, space="PSUM") as pacc:
            hsumT_ps = pacc.tile([C, 257], F32, name="hsumT_ps")
            for ci in range(NCH):
                h_ps = ph.tile([128, D], F32, name="h_ps")
                nc.tensor.matmul(h_ps[:], x_slice(0, ci), w1t8_sb[0][:],
                                 start=True, stop=False)
                nc.tensor.matmul(h_ps[:], x_slice(1, ci), w1t8_sb[1][:],
                                 start=False, stop=not b1_nonzero)
                if b1_nonzero:
                    nc.tensor.matmul(h_ps[:], ones_row[:], b1r_sb[:],
                                     start=False, stop=True)
                h_sb = hpool.tile([128, 257], F16, name="h_sb")
                nc.vector.memset(h_sb[:, 256:257], 1.0)
                nc.scalar.activation(h_sb[:, 0:256], h_ps[:], AF.Relu)
                oh = ohpool.tile([128, C], F16, name="oh")
                nc.vector.tensor_scalar(oh[:], iota_f[:], lab_sb[:, ci:ci + 1],
                                        None, OP.is_equal)
                nc.tensor.matmul(hsumT_ps[:], oh[:], h_sb[:],
                                 start=ci == 0, stop=ci == NCH - 1)
            # partial[0:100, 0:257] = [class sums | counts], fp16
            nc.scalar.copy(partial[0:100, 0:257], hsumT_ps[:])

        # ========== cross-core reduce: one-shot XOR-slotted exchange =====
        # call j sends the partial to tpb own^j, landing in the receiver's
        # slot j; the 7 calls are spread over the 4 SWDGE queues so the
        # descriptor generation (the cost driver: 16 lanes x 64 line-descs
        # per call, idle lanes emit dummies) runs in parallel.
        hsum_f = sb.tile([128, SLOT], F32, name="hsum_f")
        if exchange == "rdma":
            for j in range(1, NCORES):
                rd = [None] * NCORES
                rd[j] = (0, j)
                nc.gpsimd.remote_dma_broadcast(
                    slots[:, j:j + 1, :].squeeze(1), partial[:],
                    remote_sem=rsem, local_sem=lsem, rdests=rd)
            nc.gpsimd.trigger_dma(count=None).then_inc(tsem, 1)
            # arrival gate on SYNC (pinned after our own trigger via tsem
            # so the scheduler cannot hoist it ahead of the sends)
            stage_sem = nc.alloc_semaphore("rdma_st")
            ws = nc.sync.wait_ge(rsem, sync_wreg)
            ws.wait_op(tsem, 1, "sem-ge")
            ws.then_inc(stage_sem, 1)
            nc.vector.tensor_copy(slots[:, 0:1, :].squeeze(1), partial[:])
        else:
            arin = dram.tile([128, SLOT], F16, name="arin")
            arout = dram.tile([NCORES * 128, SLOT], F16, name="arout")
            nc.sync.dma_start(arin[:], partial[:])
            nc.gpsimd.collective_compute(
                "AllGather", OP.bypass,
                replica_groups=[list(range(NCORES))],
                ins=[arin.opt()], outs=[arout.opt()])
            ar_view = arout.rearrange("(r p) j -> p r j", p=128)
            nc.sync.dma_start(slots[:], ar_view)

        # 8-way tree sum on DVE
        a4 = sb.tile([128, 4, SLOT], F32, name="a4")
        add1 = nc.vector.tensor_tensor(a4[:], slots[:, 0:4, :],
                                       slots[:, 4:8, :], OP.add)
        if exchange == "rdma":
            add1.wait_op(stage_sem, 1, "sem-ge")
        a2v = sb.tile([128, 2, SLOT], F32, name="a2v")
        nc.vector.tensor_tensor(a2v[:], a4[:, 0:2, :], a4[:, 2:4, :], OP.add)
        nc.vector.tensor_tensor(hsum_f[:], a2v[:, 0:1, :].squeeze(1),
                                a2v[:, 1:2, :].squeeze(1), OP.add)

        # ================= query projector (runs while exchange is in
        # flight; also keeps the PE clock warm) =================
        hq_sb = [sb.tile([128, BL], F16, name=f"hq{k}") for k in range(2)]
        qt_sb = [sb.tile([128, BL], F16, name=f"qt{k}") for k in range(2)]
        qsq_sb = [sb.tile([128, BL], F16, name=f"qsq{k}") for k in range(2)]
        qaug = sb.tile([2, BL], F16, name="qaug")
        with tc.tile_pool(name="pq", bufs=2, space="PSUM") as pq:
            for qb in range(2):
                qsl = slice(qb * 512, (qb + 1) * 512)
                for mc in range(2):
                    msl = slice(mc * 128, (mc + 1) * 128)
                    hq_ps = pq.tile([128, 512], F32, name="hq_ps")
                    for kc in range(2):
                        nc.tensor.matmul(hq_ps[:], w1t_sb[kc][:, msl],
                                         xqt_sb[kc][:, qsl],
                                         start=kc == 0, stop=kc == 1)
                    nc.scalar.activation(hq_sb[mc][:, qsl], hq_ps[:], AF.Relu,
                                         bias=b1c_sb[mc][:])
                for mc in range(2):
                    msl = slice(mc * 128, (mc + 1) * 128)
                    qt_ps = pq.tile([128, 512], F32, name="qt_ps")
                    for kc in range(2):
                        nc.tensor.matmul(qt_ps[:], w2t_sb[kc][:, msl],
                                         hq_sb[kc][:, qsl],
                                         start=kc == 0, stop=kc == 1)
                    nc.scalar.copy(qt_sb[mc][:, qsl], qt_ps[:])
                    nc.scalar.square(qsq_sb[mc][:, qsl], qt_ps[:])
                # qaug rows: 0 = ||q||^2, 1 = ones
                qn_ps = pq.tile([2, 512], F32, name="qn_ps")
                for kc in range(2):
                    nc.tensor.matmul(qn_ps[:], oc2[:], qsq_sb[kc][:, qsl],
                                     start=kc == 0, stop=False)
                nc.tensor.matmul(qn_ps[:], e01[:], ones_q[:, qsl],
                                 start=False, stop=True)
                nc.scalar.copy(qaug[:, qsl], qn_ps[:])

        # ---- modest PE keep-warm filler during the exchange wait
        with tc.tile_pool(name="pfill", bufs=1, space="PSUM") as pfill:
            fill_ps = pfill.tile([128, 512], F32, name="fill_ps")
            for _ in range(16):
                nc.tensor.matmul(fill_ps[:], scratch[:, 0:128], scratch[:],
                                 start=True, stop=True, skip_group_check=True)

        # ================= prototypes =================
        cmax = sb.tile([128, 1], F32, name="cmax")
        inv = sb.tile([128, 1], F32, name="inv")
        hs_sc = sb.tile([128, D], F16, name="hs_sc")
        hsF_sb = [sb.tile([128, C], F16, name=f"hsF{k}") for k in range(2)]
        p2t_sb = [sb.tile([128, C], F16, name=f"p2t{k}") for k in range(2)]
        p2sq_sb = [sb.tile([128, C], F16, name=f"p2sq{k}") for k in range(2)]
        paug = sb.tile([2, C], F16, name="paug")
        with tc.tile_pool(name="pp", bufs=2, space="PSUM") as pp:
            nc.vector.tensor_scalar_max(cmax[0:100, :],
                                        hsum_f[0:100, 256:257], 1.0)
            nc.vector.reciprocal(inv[0:100, :], cmax[0:100, :])
            nc.vector.tensor_scalar(hs_sc[0:100, :], hsum_f[0:100, 0:256],
                                    inv[0:100, :], None, OP.mult)
            for k in range(2):
                tp_ps = pp.tile([128, C], F32, name="tp_ps")
                nc.tensor.matmul(tp_ps[:],
                                 hs_sc[0:100, k * 128:(k + 1) * 128],
                                 ident[0:100, :], start=True, stop=True)
                nc.scalar.copy(hsF_sb[k][:], tp_ps[:])
            for mc in range(2):
                msl = slice(mc * 128, (mc + 1) * 128)
                pt_ps = pp.tile([128, C], F32, name="pt_ps")
                for kc in range(2):
                    nc.tensor.matmul(pt_ps[:], w2t_sb[kc][:, msl],
                                     hsF_sb[kc][:],
                                     start=kc == 0, stop=kc == 1)
                nc.scalar.activation(p2t_sb[mc][:], pt_ps[:], AF.Copy,
                                     scale=2.0)
                nc.scalar.square(p2sq_sb[mc][:], pt_ps[:])
            # paug rows: 0 = -ones, 1 = -||p||^2
            pn_ps = pp.tile([2, C], F32, name="pn_ps")
            for mc in range(2):
                nc.tensor.matmul(pn_ps[:], zm1[:], p2sq_sb[mc][:],
                                 start=mc == 0, stop=False)
            nc.tensor.matmul(pn_ps[:], em1[:], ones_c[:],
                             start=False, stop=True)
            nc.scalar.copy(paug[:], pn_ps[:])

        # ================= distances =================
        with tc.tile_pool(name="pg", bufs=3, space="PSUM") as pg:
            for ci in range(QCH):
                csl = slice(ci * 128, (ci + 1) * 128)
                g_ps = pg.tile([128, C], F32, name="g_ps")
                nc.tensor.matmul(g_ps[:], qt_sb[0][:, csl], p2t_sb[0][:],
                                 start=True, stop=False)
                nc.tensor.matmul(g_ps[:], qt_sb[1][:, csl], p2t_sb[1][:],
                                 start=False, stop=False)
                nc.tensor.matmul(g_ps[:], qaug[:, csl], paug[:],
                                 start=False, stop=True)
                o_sb = opool.tile([128, C], F32, name="o_sb")
                nc.scalar.copy(o_sb[:], g_ps[:])
                nc.sync.dma_start(out[csl, :], o_sb[:])

    nc.compile()
    return nc


def make_in_maps(query, support_feats, support_labels, W1, b1, W2, b2):
    q = np.ascontiguousarray(np.asarray(query, dtype=np.float32))
    x = np.asarray(support_feats, dtype=np.float32)
    labels = np.asarray(support_labels).astype(np.int64)
    W1 = np.asarray(W1, dtype=np.float32)
    b1 = np.asarray(b1, dtype=np.float32)
    W2 = np.asarray(W2, dtype=np.float32)

    w1t_h = np.ascontiguousarray(W1.T).astype(np.float16)
    w1t8_h = np.ascontiguousarray(W1.T).astype(ml_dtypes.float8_e4m3fn)
    w2t_h = np.ascontiguousarray(W2.T).astype(np.float16)
    b1col = np.ascontiguousarray(b1.reshape(D, 1))
    b1row = np.ascontiguousarray(b1.reshape(1, D)).astype(np.float16)

    in_maps = []
    for c in range(NCORES):
        xs = x[c * SL:(c + 1) * SL]
        ls = labels[c * SL:(c + 1) * SL]
        xqs = q[c * BL:(c + 1) * BL]
        in_maps.append({
            "xt_sup": np.ascontiguousarray(xs.T).astype(
                ml_dtypes.float8_e4m3fn),
            "lab": np.ascontiguousarray(
                ls.reshape(NCH, 128).T.astype(np.float32)),
            "xqt": np.ascontiguousarray(xqs.T).astype(np.float16),
            "w1t": w1t_h,
            "w1t8": w1t8_h,
            "w2t": w2t_h,
            "b1c": b1col,
            "b1r": b1row,
        })
    return in_maps


EXCHANGE = os.environ.get("PROTO_EXCHANGE", "ccag")

_cached = {}


def _get_program(b1_nonzero: bool, exchange: str = None):
    key = (bool(b1_nonzero), exchange or EXCHANGE)
    if key not in _cached:
        _cached[key] = build_program(*key)
    return _cached[key]


def kernel(query, support_feats, support_labels, W1, b1, W2, b2,
           **run_kwargs):
    b1_nonzero = bool(np.any(np.asarray(b1)))
    nc = _get_program(b1_nonzero)
    in_maps = make_in_maps(query, support_feats, support_labels,
                           W1, b1, W2, b2)
    res = bass_utils.run_bass_kernel_spmd(
        nc, in_maps, core_ids=list(range(NCORES)), **run_kwargs)
    out = np.concatenate([res.results[c]["out"] for c in range(NCORES)],
                         axis=0)
    return out.astype(np.float32, copy=False)


if __name__ == "__main__":
    import sys
    sys.path.insert(0, "/root/problem")
    from reference import setup_inputs
    inputs = {k: np.asarray(v) for k, v in setup_inputs().items()}
    o = kernel(**inputs)
    print("out", o.shape, o.dtype, o[:2, :4])
